# revision 11
# baseline (speedup 1.0000x reference)
"""GQA attention (B=2, S=2048, D=2048, H=16, KV=4, HD=128) with RoPE + causal
softmax + output projection, on 8 TRN2 NeuronCores.

Sharding: B x KV = 2 x 4 = 8 perfectly balanced shards. Core c handles batch
c//4 and kv-group c%4 (4 q heads + 1 kv head). wq/wk/wv are column-sharded,
wo row-sharded; the 4 partial wo outputs per batch are summed on the host
(the unshard step for a row-sharded matmul).

v2: full-bf16 matmul pipeline (f32 PSUM accumulation, f32 output):
  - x cast to bf16 on host; x^T tiles via hardware DMA transpose (no PE
    transposes, no PSUM->SBUF copy traffic for x^T).
  - all matmul operands bf16 -> FWL weight loads (4x faster than fp32) and
    dense PE activity that keeps the HAM clock at 2.4 GHz.
  - activations stay transposed [feature, seq]: projections -> RoPE (DVE
    multiplies from PSUM + GpSimd add/sub) -> scores^T -> exp on ACT (bf16
    out, no max subtraction: scores are O(1) by construction) -> causal mask
    multiply on GpSimd -> l via all-ones [128,128] matmul (broadcasts
    sum_k P into all partitions, PSUM-accumulated) and O^T = V.T @ P^T ->
    normalize O^T by 1/l (reciprocal_approx_fast straight from PSUM) ->
    out = O^T.T @ wo accumulated over heads.
"""
import os
import sys

import numpy as np

if "/opt/trn_rl_repo" not in sys.path:
    sys.path.insert(0, "/opt/trn_rl_repo")

B, S, D = 2, 2048, 2048
H, KV, HD = 16, 4, 128
NREP = H // KV            # 4 q heads per core
EG = NREP * HD            # 512: per-core q width
NC_CORES = 8
SB = 4                    # seq blocks of 512
ST = 4                    # 128-row seq tiles per block
DT = D // 128             # 16 contraction tiles
SCALE = float(1.0 / np.sqrt(HD))

_CACHE = {}
LAST_RESULT = None        # BassKernelResults of the most recent run (for test.py)


def _install_trace_shim():
    """antenv.axon_hooks is missing in this image; run_bass_kernel_spmd's
    trace path needs it. Also neuter the S3 artifact upload."""
    import types

    try:
        import antenv.axon_hooks  # noqa: F401
    except ImportError:
        try:
            import antenv
            from trn_agent_boot.trn_boot import _ntff_profile_via_ctypes

            mod = types.ModuleType("antenv.axon_hooks")
            _hook = [None]
            mod.set_axon_ntff_profile_hook = lambda h: _hook.__setitem__(0, h)
            mod.get_axon_ntff_profile_hook = lambda: _hook[0]
            sys.modules["antenv.axon_hooks"] = mod
            antenv.axon_hooks = mod
            mod.set_axon_ntff_profile_hook(
                _ntff_profile_via_ctypes("/opt/axon/libaxon_pjrt.so")
            )
        except Exception:
            return
    import concourse.bass_utils as bu

    bu.upload_artifacts = lambda tmpdir: f"local:{tmpdir}"


def _build():
    import concourse.mybir as mybir
    import concourse.tile as tile
    from concourse import bacc

    f32 = mybir.dt.float32
    bf16 = mybir.dt.bfloat16
    EXP = mybir.ActivationFunctionType.Exp

    nc = bacc.Bacc(None, target_bir_lowering=False)
    x_d = nc.declare_dram_parameter("x", [S, D], bf16, isOutput=False)
    wq_d = nc.declare_dram_parameter("wq", [D, EG], bf16, isOutput=False)
    wk_d = nc.declare_dram_parameter("wk", [D, HD], bf16, isOutput=False)
    wv_d = nc.declare_dram_parameter("wv", [D, HD], bf16, isOutput=False)
    wo_d = nc.declare_dram_parameter("wo", [EG, D], bf16, isOutput=False)
    cs_d = nc.declare_dram_parameter("csT", [128, S], f32, isOutput=False)
    g_d = nc.declare_dram_parameter("gmask", [128, 896], bf16, isOutput=False)
    ones_d = nc.declare_dram_parameter("ones", [128, 128], bf16, isOutput=False)
    out_d = nc.declare_dram_parameter("out", [S, D], f32, isOutput=True)

    with tile.TileContext(nc) as tc:
        with (
            tc.tile_pool(name="fixed", bufs=1) as fixed,
            tc.tile_pool(name="xt", bufs=4) as xtp,
            tc.tile_pool(name="qt", bufs=2) as qtp,
            tc.tile_pool(name="ot", bufs=2) as otp,
            tc.tile_pool(name="pt", bufs=4) as ptp,
            tc.tile_pool(name="rope", bufs=3) as ropep,
            tc.tile_pool(name="cs", bufs=4) as csp,
            tc.tile_pool(name="vt", bufs=2) as vtp,
            tc.tile_pool(name="r", bufs=2) as rp,
            tc.tile_pool(name="ob", bufs=3) as obp,
            # PSUM banks: proj(2) + o(2) + st(3) + l(1) = 8
            tc.tile_pool(name="psA", bufs=2, space="PSUM") as psA,
            tc.tile_pool(name="psS", bufs=3, space="PSUM") as psS,
            tc.tile_pool(name="psB", bufs=1, space="PSUM") as psB,
        ):
            # ---- persistent tiles (DMAs emitted in the ordered prologue) ----
            wq_s = fixed.tile([128, DT, EG], bf16)
            wk_s = fixed.tile([128, DT, HD], bf16)
            wv_s = fixed.tile([128, DT, HD], bf16)
            wo_s = fixed.tile([128, NREP, D], bf16)
            g_s = fixed.tile([128, 896], bf16)
            # all-ones [128,128]: the l-matmul broadcasts sum_k P into every
            # output partition at the same cost as an M=1 matmul (cost ~ N),
            # making 1/l directly consumable by the O^T normalize multiply.
            ones_s = fixed.tile([128, 128], bf16)
            KT = fixed.tile([128, SB, 512], bf16)   # rotated K^T [hd, s]
            V = fixed.tile([128, DT, HD], bf16)     # V [s%128, s-tile, hd]

            def rope(dst, psrc, cs):
                """dst[128,512] bf16 = rotate(psrc[128,512] PSUM f32).
                Rows 0:64 = real half, 64:128 = imag half (pre-permuted
                weights); cs rows 0:64 = cos^T, 64:128 = sin^T. Multiplies
                on DVE (PSUM reads), add/sub on GpSimd (SBUF only)."""
                re, im = psrc[0:64, :], psrc[64:128, :]
                co, si = cs[0:64, :], cs[64:128, :]
                t1 = ropep.tile([64, 512], bf16, tag="t1")
                nc.vector.tensor_mul(t1, re, co)
                t2 = ropep.tile([64, 512], bf16, tag="t2")
                nc.vector.tensor_mul(t2, im, si)
                nc.gpsimd.tensor_sub(dst[0:64, :], t1, t2)
                t3 = ropep.tile([64, 512], bf16, tag="t1")
                nc.vector.tensor_mul(t3, re, si)
                t4 = ropep.tile([64, 512], bf16, tag="t2")
                nc.vector.tensor_mul(t4, im, co)
                nc.gpsimd.tensor_add(dst[64:128, :], t3, t4)

            def load_xt(sb):
                """x^T for block sb via hardware DMA transpose. The transpose
                crossbar runs ~13 GB/s per queue, so split 16 ways to put
                every chunk on its own queue (~10us each)."""
                xt = xtp.tile([128, DT, 512], bf16, tag="xt")
                for dg in range(DT):
                    nc.sync.dma_start_transpose(
                        xt[:, dg, :],
                        x_d.ap()[
                            sb * 512 : (sb + 1) * 512,
                            dg * 128 : (dg + 1) * 128,
                        ],
                    )
                cs = csp.tile([128, 512], f32, tag="cs")
                nc.sync.dma_start(cs, cs_d.ap()[:, sb * 512 : (sb + 1) * 512])
                return xt, cs

            def stage2(sb, xt, cs):
                """Q^T/K^T/V projections + RoPE for block sb."""
                qt = qtp.tile([128, NREP, 512], bf16, tag="qt")
                for h in range(NREP):
                    pq = psA.tile([128, 512], f32, tag="proj")
                    for dt in range(DT):
                        nc.tensor.matmul(
                            pq,
                            wq_s[:, dt, h * 128 : (h + 1) * 128],
                            xt[:, dt, :],
                            start=(dt == 0),
                            stop=(dt == DT - 1),
                        )
                    rope(qt[:, h, :], pq, cs)

                pk = psA.tile([128, 512], f32, tag="proj")
                for dt in range(DT):
                    nc.tensor.matmul(
                        pk, wk_s[:, dt, :], xt[:, dt, :],
                        start=(dt == 0), stop=(dt == DT - 1),
                    )
                rope(KT[:, sb, :], pk, cs)

                pv = psA.tile([128, 512], f32, tag="proj")
                for dt in range(DT):
                    nc.tensor.matmul(
                        pv, wv_s[:, dt, :], xt[:, dt, :],
                        start=(dt == 0), stop=(dt == DT - 1),
                    )
                vt_tmp = vtp.tile([128, 512], bf16, tag="vt")
                nc.vector.tensor_copy(vt_tmp, pv)
                nc.sync.dma_start_transpose(V[:, sb * ST : (sb + 1) * ST, :], vt_tmp)
                return qt

            def stage3(sb, qt):
                """Causal attention for q-block sb, all 4 heads.
                Emission is software-pipelined: 3 score matmuls run ahead of
                the exp->mask->l/PV chain so the in-order PE stream never
                stalls on ACT/GpSimd latency."""
                ot = otp.tile([128, NREP, 512], bf16, tag="ot")
                nkt = (sb + 1) * ST
                DEPTH = 3

                def kt_geo(kt):
                    """Valid q range for k-tile kt in this q-block: diagonal
                    tiles only cover q >= 128*r (causal width trim)."""
                    r = kt - sb * ST
                    qo = 128 * r if r > 0 else 0
                    return r, qo, 512 - qo

                for h in range(NREP):
                    po = psA.tile([128, 512], f32, tag="o")
                    pl = psB.tile([128, 512], f32, tag="l")

                    def emit_st(kt):
                        r, qo, w = kt_geo(kt)
                        pst = psS.tile([128, 512], f32, tag="st")
                        nc.tensor.matmul(
                            pst[:, qo:],
                            KT[:, kt // ST, (kt % ST) * 128 : (kt % ST + 1) * 128],
                            qt[:, h, qo:],
                            start=True, stop=True,
                        )
                        return pst

                    sts = {}
                    for kt in range(min(DEPTH, nkt)):
                        sts[kt] = emit_st(kt)
                    for kt in range(nkt):
                        r, qo, w = kt_geo(kt)
                        pst = sts.pop(kt)
                        pt = ptp.tile([128, 512], bf16, tag="pt")
                        nc.scalar.activation(pt[:, qo:], pst[:, qo:], EXP, scale=SCALE)
                        if r >= 0:
                            # invalid triangle only spans the strip's first
                            # 128 columns after the causal width trim
                            nc.gpsimd.tensor_mul(
                                pt[:, qo : qo + 128],
                                pt[:, qo : qo + 128],
                                g_s[:, 384:512],
                            )
                        nc.tensor.matmul(
                            pl[:, qo:], ones_s, pt[:, qo:],
                            start=(kt == 0), stop=(kt == nkt - 1),
                        )
                        nc.tensor.matmul(
                            po[:, qo:], V[:, kt, :], pt[:, qo:],
                            start=(kt == 0), stop=(kt == nkt - 1),
                        )
                        if kt + DEPTH < nkt:
                            sts[kt + DEPTH] = emit_st(kt + DEPTH)
                    rb = rp.tile([128, 512], f32, tag="rb")
                    nc.vector.reciprocal_approx_fast(out=rb, in_=pl)
                    nc.vector.tensor_mul(ot[:, h, :], po, rb)
                return ot

            def stage4(sb, ot):
                """Output projection for q-block sb."""
                for db in range(4):
                    for st in range(ST):
                        pw = psA.tile([128, 512], f32, tag="proj")
                        for h in range(NREP):
                            nc.tensor.matmul(
                                pw,
                                ot[:, h, st * 128 : (st + 1) * 128],
                                wo_s[:, h, db * 512 : (db + 1) * 512],
                                start=(h == 0), stop=(h == NREP - 1),
                            )
                        ob = obp.tile([128, 512], f32, tag="ob")
                        nc.vector.tensor_copy(ob, pw)
                        row0 = (sb * ST + st) * 128
                        nc.sync.dma_start(
                            out_d.ap()[row0 : row0 + 128, db * 512 : (db + 1) * 512],
                            ob,
                        )

            # Software-pipelined outer loop: projections for block sb+1 are
            # emitted BEFORE the wo-stage of block sb, so the in-order PE
            # stream has independent matmuls to run while block sb's
            # normalization tail (recip + broadcast DMA) completes.
            # ---- ordered DMA prologue: block-0 x^T and wq first so
            # stage2(0) can start ~20us in; everything else behind them ----
            xts = [load_xt(0)]
            wq_ap = wq_d.ap().rearrange("(t k) e -> k t e", k=128)
            for i in range(8):
                nc.sync.dma_start(
                    wq_s[:, 2 * i : 2 * i + 2, :], wq_ap[:, 2 * i : 2 * i + 2, :]
                )
            nc.sync.dma_start(wk_s, wk_d.ap().rearrange("(t k) e -> k t e", k=128))
            nc.sync.dma_start(wv_s, wv_d.ap().rearrange("(t k) e -> k t e", k=128))
            xts.append(load_xt(1))
            nc.sync.dma_start(g_s, g_d.ap())
            nc.sync.dma_start(ones_s, ones_d.ap())
            xts.append(load_xt(2))
            xts.append(load_xt(3))
            wo_ap = wo_d.ap().rearrange("(h k) n -> k h n", k=128)
            for i in range(4):
                nc.sync.dma_start(
                    wo_s[:, :, 512 * i : 512 * i + 512],
                    wo_ap[:, :, 512 * i : 512 * i + 512],
                )
            qt = stage2(0, *xts[0])
            for sb in range(SB):
                ot = stage3(sb, qt)
                if sb + 1 < SB:
                    qt = stage2(sb + 1, *xts[sb + 1])
                stage4(sb, ot)
    nc.finalize()
    return nc


def _get_nc():
    if "nc" not in _CACHE:
        _CACHE["nc"] = _build()
    return _CACHE["nc"]


def _host_prep(x, wq, wk, wv, wo, freqs_cos, freqs_sin):
    """Build the 8 per-core input maps (bf16 casts on host)."""
    import ml_dtypes

    bf = ml_dtypes.bfloat16
    perm = np.concatenate([np.arange(0, HD, 2), np.arange(1, HD, 2)])  # even|odd
    csT = np.concatenate(
        [np.ascontiguousarray(freqs_cos.T), np.ascontiguousarray(freqs_sin.T)], axis=0
    ).astype(np.float32)  # [128, S]
    # gmask[kk, v] = 1 iff (v - 384) >= kk ; mask for rel pos r = cols 384-128r ..
    vv = np.arange(896, dtype=np.int64)[None, :] - 384
    kk = np.arange(128, dtype=np.int64)[:, None]
    gmask = (vv >= kk).astype(bf)
    ones = np.ones((128, 128), dtype=bf)

    in_maps = []
    for c in range(NC_CORES):
        b, g = divmod(c, NREP)
        wq_g = wq[:, g * EG : (g + 1) * EG].copy()
        for h in range(NREP):
            blk = wq_g[:, h * HD : (h + 1) * HD]
            wq_g[:, h * HD : (h + 1) * HD] = blk[:, perm]
        wk_g = wk[:, g * HD : (g + 1) * HD][:, perm]
        wv_g = wv[:, g * HD : (g + 1) * HD]
        wo_g = wo[g * EG : (g + 1) * EG, :]
        in_maps.append(
            {
                "x": np.ascontiguousarray(x[b]).astype(bf),
                "wq": np.ascontiguousarray(wq_g).astype(bf),
                "wk": np.ascontiguousarray(wk_g).astype(bf),
                "wv": np.ascontiguousarray(wv_g).astype(bf),
                "wo": np.ascontiguousarray(wo_g).astype(bf),
                "csT": csT,
                "gmask": gmask,
                "ones": ones,
            }
        )
    return in_maps


def kernel(x, wq, wk, wv, wo, freqs_cos, freqs_sin):
    global LAST_RESULT
    from concourse.bass_utils import run_bass_kernel_spmd

    trace = bool(int(os.environ.get("BASS_KERNEL_TRACE", "0")))
    if trace:
        _install_trace_shim()

    x = np.asarray(x, dtype=np.float32)
    wq = np.asarray(wq, dtype=np.float32)
    wk = np.asarray(wk, dtype=np.float32)
    wv = np.asarray(wv, dtype=np.float32)
    wo = np.asarray(wo, dtype=np.float32)
    freqs_cos = np.asarray(freqs_cos, dtype=np.float32)
    freqs_sin = np.asarray(freqs_sin, dtype=np.float32)

    nc = _get_nc()
    in_maps = _host_prep(x, wq, wk, wv, wo, freqs_cos, freqs_sin)
    res = run_bass_kernel_spmd(nc, in_maps, list(range(NC_CORES)), trace=trace)
    LAST_RESULT = res

    out = np.empty((B, S, D), dtype=np.float32)
    for b in range(B):
        acc = res.results[b * NREP]["out"].astype(np.float32, copy=True)
        for g in range(1, NREP):
            acc += res.results[b * NREP + g]["out"]
        out[b] = acc
    return out


# revision 12
# speedup vs baseline: 1.1519x; 1.1519x over previous
"""GQA attention (B=2, S=2048, D=2048, H=16, KV=4, HD=128) with RoPE + causal
softmax + output projection, on 8 TRN2 NeuronCores.

Sharding: B x KV = 2 x 4 = 8 perfectly balanced shards. Core c handles batch
c//4 and kv-group c%4 (4 q heads + 1 kv head). wq/wk/wv are column-sharded,
wo row-sharded; the 4 partial wo outputs per batch are summed on the host
(the unshard step for a row-sharded matmul).

v2: full-bf16 matmul pipeline (f32 PSUM accumulation, f32 output):
  - x cast to bf16 on host; x^T tiles via hardware DMA transpose (no PE
    transposes, no PSUM->SBUF copy traffic for x^T).
  - all matmul operands bf16 -> FWL weight loads (4x faster than fp32) and
    dense PE activity that keeps the HAM clock at 2.4 GHz.
  - activations stay transposed [feature, seq]: projections -> RoPE (DVE
    multiplies from PSUM + GpSimd add/sub) -> scores^T -> exp on ACT (bf16
    out, no max subtraction: scores are O(1) by construction) -> causal mask
    multiply on GpSimd -> l via all-ones [128,128] matmul (broadcasts
    sum_k P into all partitions, PSUM-accumulated) and O^T = V.T @ P^T ->
    normalize O^T by 1/l (reciprocal_approx_fast straight from PSUM) ->
    out = O^T.T @ wo accumulated over heads.
"""
import os
import sys

import numpy as np

if "/opt/trn_rl_repo" not in sys.path:
    sys.path.insert(0, "/opt/trn_rl_repo")

B, S, D = 2, 2048, 2048
H, KV, HD = 16, 4, 128
NREP = H // KV            # 4 q heads per core
EG = NREP * HD            # 512: per-core q width
NC_CORES = 8
SB = 4                    # seq blocks of 512
ST = 4                    # 128-row seq tiles per block
DT = D // 128             # 16 contraction tiles
SCALE = float(1.0 / np.sqrt(HD))

_CACHE = {}
LAST_RESULT = None        # BassKernelResults of the most recent run (for test.py)


def _install_trace_shim():
    """antenv.axon_hooks is missing in this image; run_bass_kernel_spmd's
    trace path needs it. Also neuter the S3 artifact upload."""
    import types

    try:
        import antenv.axon_hooks  # noqa: F401
    except ImportError:
        try:
            import antenv
            from trn_agent_boot.trn_boot import _ntff_profile_via_ctypes

            mod = types.ModuleType("antenv.axon_hooks")
            _hook = [None]
            mod.set_axon_ntff_profile_hook = lambda h: _hook.__setitem__(0, h)
            mod.get_axon_ntff_profile_hook = lambda: _hook[0]
            sys.modules["antenv.axon_hooks"] = mod
            antenv.axon_hooks = mod
            mod.set_axon_ntff_profile_hook(
                _ntff_profile_via_ctypes("/opt/axon/libaxon_pjrt.so")
            )
        except Exception:
            return
    import concourse.bass_utils as bu

    bu.upload_artifacts = lambda tmpdir: f"local:{tmpdir}"


def _build():
    import concourse.mybir as mybir
    import concourse.tile as tile
    from concourse import bacc

    f32 = mybir.dt.float32
    bf16 = mybir.dt.bfloat16
    EXP = mybir.ActivationFunctionType.Exp

    nc = bacc.Bacc(None, target_bir_lowering=False)
    x_d = nc.declare_dram_parameter("x", [S, D], bf16, isOutput=False)
    wq_d = nc.declare_dram_parameter("wq", [D, EG], bf16, isOutput=False)
    wk_d = nc.declare_dram_parameter("wk", [D, HD], bf16, isOutput=False)
    wv_d = nc.declare_dram_parameter("wv", [D, HD], bf16, isOutput=False)
    wo_d = nc.declare_dram_parameter("wo", [EG, D], bf16, isOutput=False)
    cs_d = nc.declare_dram_parameter("csT", [128, S], f32, isOutput=False)
    g_d = nc.declare_dram_parameter("gmask", [128, 896], bf16, isOutput=False)
    ones_d = nc.declare_dram_parameter("ones", [128, 128], bf16, isOutput=False)
    out_d = nc.declare_dram_parameter("out", [S, D], f32, isOutput=True)

    with tile.TileContext(nc) as tc:
        with (
            tc.tile_pool(name="fixed", bufs=1) as fixed,
            tc.tile_pool(name="xt", bufs=4) as xtp,
            tc.tile_pool(name="qt", bufs=2) as qtp,
            tc.tile_pool(name="ot", bufs=2) as otp,
            tc.tile_pool(name="pt", bufs=4) as ptp,
            tc.tile_pool(name="rope", bufs=3) as ropep,
            tc.tile_pool(name="cs", bufs=4) as csp,
            tc.tile_pool(name="vt", bufs=2) as vtp,
            tc.tile_pool(name="r", bufs=2) as rp,
            tc.tile_pool(name="ob", bufs=3) as obp,
            # PSUM banks: proj(2) + o(2) + st(3) + l(1) = 8
            tc.tile_pool(name="psA", bufs=2, space="PSUM") as psA,
            tc.tile_pool(name="psS", bufs=3, space="PSUM") as psS,
            tc.tile_pool(name="psB", bufs=1, space="PSUM") as psB,
        ):
            # ---- persistent tiles (DMAs emitted in the ordered prologue) ----
            wq_s = fixed.tile([128, DT, EG], bf16)
            wk_s = fixed.tile([128, DT, HD], bf16)
            wv_s = fixed.tile([128, DT, HD], bf16)
            wo_s = fixed.tile([128, NREP, D], bf16)
            g_s = fixed.tile([128, 896], bf16)
            # all-ones [128,128]: the l-matmul broadcasts sum_k P into every
            # output partition at the same cost as an M=1 matmul (cost ~ N),
            # making 1/l directly consumable by the O^T normalize multiply.
            ones_s = fixed.tile([128, 128], bf16)
            KT = fixed.tile([128, SB, 512], bf16)   # rotated K^T [hd, s]
            V = fixed.tile([128, DT, HD], bf16)     # V [s%128, s-tile, hd]

            def rope(dst, psrc, cs):
                """dst[128,512] bf16 = rotate(psrc[128,512] PSUM f32).
                Rows 0:64 = real half, 64:128 = imag half (pre-permuted
                weights); cs rows 0:64 = cos^T, 64:128 = sin^T. Multiplies
                on DVE (PSUM reads), add/sub on GpSimd (SBUF only)."""
                re, im = psrc[0:64, :], psrc[64:128, :]
                co, si = cs[0:64, :], cs[64:128, :]
                t1 = ropep.tile([64, 512], bf16, tag="t1")
                nc.vector.tensor_mul(t1, re, co)
                t2 = ropep.tile([64, 512], bf16, tag="t2")
                nc.vector.tensor_mul(t2, im, si)
                nc.gpsimd.tensor_sub(dst[0:64, :], t1, t2)
                t3 = ropep.tile([64, 512], bf16, tag="t1")
                nc.vector.tensor_mul(t3, re, si)
                t4 = ropep.tile([64, 512], bf16, tag="t2")
                nc.vector.tensor_mul(t4, im, co)
                nc.gpsimd.tensor_add(dst[64:128, :], t3, t4)

            def load_xt(sb):
                """x^T for block sb via hardware DMA transpose. The transpose
                crossbar runs ~13 GB/s per queue, so split 16 ways to put
                every chunk on its own queue (~10us each)."""
                xt = xtp.tile([128, DT, 512], bf16, tag="xt")
                for dg in range(4):
                    nc.sync.dma_start_transpose(
                        xt[:, dg * 4 : (dg + 1) * 4, :],
                        x_d.ap()[
                            sb * 512 : (sb + 1) * 512,
                            dg * 512 : (dg + 1) * 512,
                        ],
                    )
                cs = csp.tile([128, 512], f32, tag="cs")
                nc.sync.dma_start(cs, cs_d.ap()[:, sb * 512 : (sb + 1) * 512])
                return xt, cs

            def stage2(sb, xt, cs):
                """Q^T/K^T/V projections + RoPE for block sb."""
                qt = qtp.tile([128, NREP, 512], bf16, tag="qt")
                for h in range(NREP):
                    pq = psA.tile([128, 512], f32, tag="proj")
                    for dt in range(DT):
                        nc.tensor.matmul(
                            pq,
                            wq_s[:, dt, h * 128 : (h + 1) * 128],
                            xt[:, dt, :],
                            start=(dt == 0),
                            stop=(dt == DT - 1),
                        )
                    rope(qt[:, h, :], pq, cs)

                pk = psA.tile([128, 512], f32, tag="proj")
                for dt in range(DT):
                    nc.tensor.matmul(
                        pk, wk_s[:, dt, :], xt[:, dt, :],
                        start=(dt == 0), stop=(dt == DT - 1),
                    )
                rope(KT[:, sb, :], pk, cs)

                pv = psA.tile([128, 512], f32, tag="proj")
                for dt in range(DT):
                    nc.tensor.matmul(
                        pv, wv_s[:, dt, :], xt[:, dt, :],
                        start=(dt == 0), stop=(dt == DT - 1),
                    )
                vt_tmp = vtp.tile([128, 512], bf16, tag="vt")
                nc.vector.tensor_copy(vt_tmp, pv)
                nc.sync.dma_start_transpose(V[:, sb * ST : (sb + 1) * ST, :], vt_tmp)
                return qt

            def stage3(sb, qt):
                """Causal attention for q-block sb, all 4 heads.
                Emission is software-pipelined: 3 score matmuls run ahead of
                the exp->mask->l/PV chain so the in-order PE stream never
                stalls on ACT/GpSimd latency."""
                ot = otp.tile([128, NREP, 512], bf16, tag="ot")
                nkt = (sb + 1) * ST
                DEPTH = 3

                def kt_geo(kt):
                    """Valid q range for k-tile kt in this q-block: diagonal
                    tiles only cover q >= 128*r (causal width trim)."""
                    r = kt - sb * ST
                    qo = 128 * r if r > 0 else 0
                    return r, qo, 512 - qo

                for h in range(NREP):
                    po = psA.tile([128, 512], f32, tag="o")
                    pl = psB.tile([128, 512], f32, tag="l")

                    def emit_st(kt):
                        r, qo, w = kt_geo(kt)
                        pst = psS.tile([128, 512], f32, tag="st")
                        nc.tensor.matmul(
                            pst[:, qo:],
                            KT[:, kt // ST, (kt % ST) * 128 : (kt % ST + 1) * 128],
                            qt[:, h, qo:],
                            start=True, stop=True,
                        )
                        return pst

                    sts = {}
                    for kt in range(min(DEPTH, nkt)):
                        sts[kt] = emit_st(kt)
                    for kt in range(nkt):
                        r, qo, w = kt_geo(kt)
                        pst = sts.pop(kt)
                        pt = ptp.tile([128, 512], bf16, tag="pt")
                        nc.scalar.activation(pt[:, qo:], pst[:, qo:], EXP, scale=SCALE)
                        if r >= 0:
                            # invalid triangle only spans the strip's first
                            # 128 columns after the causal width trim
                            nc.gpsimd.tensor_mul(
                                pt[:, qo : qo + 128],
                                pt[:, qo : qo + 128],
                                g_s[:, 384:512],
                            )
                        nc.tensor.matmul(
                            pl[:, qo:], ones_s, pt[:, qo:],
                            start=(kt == 0), stop=(kt == nkt - 1),
                        )
                        nc.tensor.matmul(
                            po[:, qo:], V[:, kt, :], pt[:, qo:],
                            start=(kt == 0), stop=(kt == nkt - 1),
                        )
                        if kt + DEPTH < nkt:
                            sts[kt + DEPTH] = emit_st(kt + DEPTH)
                    rb = rp.tile([128, 512], f32, tag="rb")
                    nc.vector.reciprocal_approx_fast(out=rb, in_=pl)
                    nc.vector.tensor_mul(ot[:, h, :], po, rb)
                return ot

            def stage4(sb, ot):
                """Output projection for q-block sb."""
                for db in range(4):
                    for st in range(ST):
                        pw = psA.tile([128, 512], f32, tag="proj")
                        for h in range(NREP):
                            nc.tensor.matmul(
                                pw,
                                ot[:, h, st * 128 : (st + 1) * 128],
                                wo_s[:, h, db * 512 : (db + 1) * 512],
                                start=(h == 0), stop=(h == NREP - 1),
                            )
                        ob = obp.tile([128, 512], f32, tag="ob")
                        nc.vector.tensor_copy(ob, pw)
                        row0 = (sb * ST + st) * 128
                        nc.sync.dma_start(
                            out_d.ap()[row0 : row0 + 128, db * 512 : (db + 1) * 512],
                            ob,
                        )

            # Software-pipelined outer loop: projections for block sb+1 are
            # emitted BEFORE the wo-stage of block sb, so the in-order PE
            # stream has independent matmuls to run while block sb's
            # normalization tail (recip + broadcast DMA) completes.
            # ---- ordered DMA prologue: block-0 x^T and wq first so
            # stage2(0) can start ~20us in; everything else behind them ----
            xts = [load_xt(0)]
            wq_ap = wq_d.ap().rearrange("(t k) e -> k t e", k=128)
            for i in range(8):
                nc.sync.dma_start(
                    wq_s[:, 2 * i : 2 * i + 2, :], wq_ap[:, 2 * i : 2 * i + 2, :]
                )
            nc.sync.dma_start(wk_s, wk_d.ap().rearrange("(t k) e -> k t e", k=128))
            nc.sync.dma_start(wv_s, wv_d.ap().rearrange("(t k) e -> k t e", k=128))
            xts.append(load_xt(1))
            nc.sync.dma_start(g_s, g_d.ap())
            nc.sync.dma_start(ones_s, ones_d.ap())
            xts.append(load_xt(2))
            xts.append(load_xt(3))
            wo_ap = wo_d.ap().rearrange("(h k) n -> k h n", k=128)
            for i in range(4):
                nc.sync.dma_start(
                    wo_s[:, :, 512 * i : 512 * i + 512],
                    wo_ap[:, :, 512 * i : 512 * i + 512],
                )
            qt = stage2(0, *xts[0])
            for sb in range(SB):
                ot = stage3(sb, qt)
                if sb + 1 < SB:
                    qt = stage2(sb + 1, *xts[sb + 1])
                stage4(sb, ot)
    nc.finalize()
    return nc


def _get_nc():
    if "nc" not in _CACHE:
        _CACHE["nc"] = _build()
    return _CACHE["nc"]


def _host_prep(x, wq, wk, wv, wo, freqs_cos, freqs_sin):
    """Build the 8 per-core input maps (bf16 casts on host)."""
    import ml_dtypes

    bf = ml_dtypes.bfloat16
    perm = np.concatenate([np.arange(0, HD, 2), np.arange(1, HD, 2)])  # even|odd
    csT = np.concatenate(
        [np.ascontiguousarray(freqs_cos.T), np.ascontiguousarray(freqs_sin.T)], axis=0
    ).astype(np.float32)  # [128, S]
    # gmask[kk, v] = 1 iff (v - 384) >= kk ; mask for rel pos r = cols 384-128r ..
    vv = np.arange(896, dtype=np.int64)[None, :] - 384
    kk = np.arange(128, dtype=np.int64)[:, None]
    gmask = (vv >= kk).astype(bf)
    ones = np.ones((128, 128), dtype=bf)

    in_maps = []
    for c in range(NC_CORES):
        b, g = divmod(c, NREP)
        wq_g = wq[:, g * EG : (g + 1) * EG].copy()
        for h in range(NREP):
            blk = wq_g[:, h * HD : (h + 1) * HD]
            wq_g[:, h * HD : (h + 1) * HD] = blk[:, perm]
        wk_g = wk[:, g * HD : (g + 1) * HD][:, perm]
        wv_g = wv[:, g * HD : (g + 1) * HD]
        wo_g = wo[g * EG : (g + 1) * EG, :]
        in_maps.append(
            {
                "x": np.ascontiguousarray(x[b]).astype(bf),
                "wq": np.ascontiguousarray(wq_g).astype(bf),
                "wk": np.ascontiguousarray(wk_g).astype(bf),
                "wv": np.ascontiguousarray(wv_g).astype(bf),
                "wo": np.ascontiguousarray(wo_g).astype(bf),
                "csT": csT,
                "gmask": gmask,
                "ones": ones,
            }
        )
    return in_maps


def kernel(x, wq, wk, wv, wo, freqs_cos, freqs_sin):
    global LAST_RESULT
    from concourse.bass_utils import run_bass_kernel_spmd

    trace = bool(int(os.environ.get("BASS_KERNEL_TRACE", "0")))
    if trace:
        _install_trace_shim()

    x = np.asarray(x, dtype=np.float32)
    wq = np.asarray(wq, dtype=np.float32)
    wk = np.asarray(wk, dtype=np.float32)
    wv = np.asarray(wv, dtype=np.float32)
    wo = np.asarray(wo, dtype=np.float32)
    freqs_cos = np.asarray(freqs_cos, dtype=np.float32)
    freqs_sin = np.asarray(freqs_sin, dtype=np.float32)

    nc = _get_nc()
    in_maps = _host_prep(x, wq, wk, wv, wo, freqs_cos, freqs_sin)
    res = run_bass_kernel_spmd(nc, in_maps, list(range(NC_CORES)), trace=trace)
    LAST_RESULT = res

    out = np.empty((B, S, D), dtype=np.float32)
    for b in range(B):
        acc = res.results[b * NREP]["out"].astype(np.float32, copy=True)
        for g in range(1, NREP):
            acc += res.results[b * NREP + g]["out"]
        out[b] = acc
    return out
